# revision 2
# baseline (speedup 1.0000x reference)
"""Trainium2 Bass kernel for nn_DFE_model (gnn_message_passing).

Math: reference scatters upd[m,i] = A_vals[i]*X[m, A_cols[i]//2] -
V[A_rows[i], A_cols[i]] into D[m, :, :] (last write wins per (row, col)),
then H[m] = sum_j F[j] * exp(-sum_k W[j,k]*relu(D[m,j,k])^2).

Per active slot s (j, k, f=k//2) with P = sqrt(W)*a, Q = sqrt(W)*V, the
contribution to E[j, m] is relu(P*x[m,f] - Q)^2.  Key layout idea: keep
X^T resident in SBUF as a few [128, 512] "feature tiles" (feature on the
partition axis) and express each batch of 128 slots (a "round") as one
per-partition affine+relu+square over a resident tile, followed by a
[128 slot -> 64 j] mask matmul accumulating E in PSUM.  No gathered
[slot, m] tensor ever leaves HBM, so DMA drops from ~2.3 MB/core to
~0.7 MB/core.

Slot classes (host, exact, using the actual per-feature min/max of X):
  - sure-zero: relu is 0 for every m -> dropped.
  - wild always-on (|t|=|Q/P| > 100 or P==0): relu-free rounds with
    v = P*x - Q (ts mult,add), mask 1.0.
  - tame (the rest): sign-homogeneous rounds: u = max(x - t, 0) for
    P > 0 rounds, u = min(x - t, 0) for P < 0 (square kills the sign),
    with P^2 folded into the fp16 mask weights.
Rounds are packed onto 3 resident tiles: identity (features 0..127) and
one spill tile per sign whose cells are allocated to overflowing
features.  Engine schedule is static: relu on DVE (tensor_scalar, fast
2x/4x modes) or ACT (activation Relu with per-partition scale/bias);
square on DVE tensor_tensor / ACT Square / Pool(gpsimd) tensor_tensor.
Epilogue on device: delta = exp(-E) (ACT, from PSUM), H_c = F_c^T @
delta (PE), 2 KB DMA out per core; host sums the 8 partials.
"""

import numpy as np

import concourse.bass as bass
import concourse.mybir as mybir
import concourse.tile as tile
from concourse.bass_utils import run_bass_kernel_spmd

# ---------------------------------------------------------------- constants
M = 512
J = 512
K = 256
NF = 128            # features = K//2
NCORES = 8
JC = J // NCORES

_DT = mybir.dt.float32
_DT16 = mybir.dt.float16
_NP16 = np.float16

WILD_T = 100.0      # |t| above this -> relu-free (always-on) round form

# Round schedule types
OP_MAX, OP_MIN, OP_FREE = 0, 1, 2          # DVE ts op / ACT scale form
# engine paths: (relu_engine, square_engine)
PATH_B = ("dve", "dve")
PATH_A = ("act", "dve")
PATH_Q = ("dve", "pool")
PATH_P = ("act", "pool")
PATH_C = ("act", "act")


# ------------------------------------------------------- walrus wait limit
def _legalize_waits(nc, max_waits=1):
    """This walrus build accepts only one sem-wait command per instruction.
    Tile emits up to ~3; move extras onto same-engine NoOps."""
    n = 0
    for f in nc.m.functions:
        for b in f.blocks:
            out, changed = [], False
            for inst in list(b.instructions):
                si = inst.sync_info
                waits = list(si.on_wait) if si and si.on_wait else []
                if len(waits) > max_waits:
                    for w in waits[max_waits:]:
                        n += 1
                        nop = mybir.InstNoOp(name=f"waitfix_{n}", ins=[], outs=[])
                        nop.engine = inst.engine
                        nop.sync_info = mybir.SyncInfo(on_wait=[w], on_update=[])
                        out.append(nop)
                    si.on_wait = waits[:max_waits]
                    changed = True
                out.append(inst)
            if changed:
                b.instructions = out


# ------------------------------------------------ slim Tile exit barrier
def _slim_drain_and_barrier(self, tick_clock, wait_clock):
    from concourse.vector_clock import ScopedClock

    drain_sp = self.nc.sync.drain()
    wait_clock.add_sem_waits(
        drain_sp.ins, ScopedClock({None: tick_clock.global_clock})
    )
    drain_gp = self.nc.gpsimd.drain()
    wait_clock.add_sem_waits(
        drain_gp.ins, ScopedClock({None: tick_clock.global_clock})
    )
    assert self.sems is not None
    popped = self.nc._tile_sem_poison_stack.pop()
    assert popped is self._sem_poison
    self.nc.clear_and_free_semaphores(list(self.sems.allocated().values()))


tile.TileContext._drain_and_barrier = _slim_drain_and_barrier


# ---------------------------------------------------------------- packing
def _prepare(X, A_vals, V, W, Fvec, A_rows, A_cols):
    """Returns (schedule, in_maps).  schedule = dict with R, per-round
    (tile, op, path); in_maps = per-core input arrays."""
    rows = np.asarray(A_rows).astype(np.int64)
    cols = np.asarray(A_cols).astype(np.int64)
    X = np.asarray(X, dtype=np.float32)
    A_vals = np.asarray(A_vals, dtype=np.float32)
    V = np.asarray(V, dtype=np.float32)
    W = np.asarray(W, dtype=np.float32)
    Fvec = np.asarray(Fvec, dtype=np.float32)

    nnz = rows.shape[0]
    lin = rows * K + cols
    winner = np.full(J * K, -1, dtype=np.int64)
    winner[lin] = np.arange(nnz)             # duplicates: LAST write wins
    active = np.nonzero(winner >= 0)[0]
    i = winner[active]
    j = active // K
    k = active % K
    f = k // 2
    s = np.sqrt(W[j, k]).astype(np.float32)
    P = s * A_vals[i]
    Q = s * V[j, k]

    xmin = X.min(axis=0)                     # per-feature [128]
    xmax = X.max(axis=0)
    with np.errstate(divide="ignore", invalid="ignore"):
        t = np.where(P != 0, Q / np.where(P == 0, 1.0, P), 0.0)
    pos = P > 0
    neg = P < 0
    zer = P == 0
    sure_zero = (
        (pos & (t >= xmax[f])) | (neg & (t <= xmin[f])) | (zer & (Q >= 0))
    )
    sure_on = (
        (pos & (t <= xmin[f])) | (neg & (t >= xmax[f])) | (zer & (Q < 0))
    )
    keep = ~sure_zero
    wild = keep & sure_on & (zer | (np.abs(t) > WILD_T))
    tame = keep & ~wild

    core = j // JC
    jl = j % JC

    # ---- per-core per-feature tame counts by sign
    npos = np.zeros((NCORES, NF), np.int64)
    nneg = np.zeros((NCORES, NF), np.int64)
    nwld = np.zeros((NCORES, NF), np.int64)
    for c in range(NCORES):
        cs = core == c
        npos[c] = np.bincount(f[cs & tame & pos], minlength=NF)
        nneg[c] = np.bincount(f[cs & tame & neg], minlength=NF)
        nwld[c] = np.bincount(f[cs & wild], minlength=NF)

    def spill_ok(n_cf, RI, RS):
        ov = np.maximum(0, n_cf - RI)
        if RS == 0:
            return not np.any(ov > 0)
        return np.ceil(ov / RS).sum() <= NF

    def search(n_all):
        best = None
        for RI in range(0, 20):
            for RS in range(0, 10):
                if best is not None and RI + RS >= best[0] + best[1]:
                    continue
                if all(spill_ok(n_all[c], RI, RS) for c in range(NCORES)):
                    best = (RI, RS)
        return best

    RpI, RpS = search(npos)
    RmI, RmS = search(nneg)
    Rw = int(max(1, nwld.max())) if nwld.sum() else 0

    # ---------------- global round schedule: (tile, op) per round
    # tiles: 0 = identity, 1 = spill+, 2 = spill-
    rounds = []
    rounds += [(0, OP_MAX)] * RpI
    rounds += [(0, OP_MIN)] * RmI
    rounds += [(0, OP_FREE)] * Rw
    rounds += [(1, OP_MAX)] * RpS
    rounds += [(2, OP_MIN)] * RmS
    R = len(rounds)

    # Interleave order so DVE/ACT/Pool all stream from round 0 and
    # identity-tile rounds lead (their data lands first).  Keep identity
    # first: reorder only within the identity block and spill block.
    order = list(range(R))
    id_rounds = [q for q in order if rounds[q][0] == 0]
    sp_rounds = [q for q in order if rounds[q][0] != 0]
    # alternate max/min/free inside each block for engine diversity
    def _mix(lst):
        by_op = {}
        for q in lst:
            by_op.setdefault(rounds[q][1], []).append(q)
        out = []
        keys = sorted(by_op)
        while any(by_op[kk] for kk in keys):
            for kk in keys:
                if by_op[kk]:
                    out.append(by_op[kk].pop(0))
        return out
    order = _mix(id_rounds) + _mix(sp_rounds)
    rounds = [rounds[q] for q in order]

    # ---------------- engine path assignment (static, global)
    # balance: DVE = 598*b + 333*a + 265*q ; ACT = 613*(a+p) + 1226*c ;
    # Pool = 1016*(q+p).  For R ~ 16: q=5, a=9, b=R-14.
    n_q = min(5, R)
    n_a = min(9, R - n_q)
    paths = []
    qi = ai = 0
    for r in range(R):
        # spread Q rounds evenly, A rounds on the rest, B for leftovers
        if qi * R <= r * n_q and qi < n_q:
            paths.append(PATH_Q)
            qi += 1
        elif ai < n_a:
            paths.append(PATH_A)
            ai += 1
        else:
            paths.append(PATH_B)
    schedule = {"R": R, "rounds": rounds, "paths": paths}

    # ---------------- per-core data
    in_maps = []
    for c in range(NCORES):
        cs = core == c
        # spill cell maps
        def cells_for(n_cf, RI, RS):
            ov = np.maximum(0, n_cf - RI)
            cmap = []            # partition -> feature
            if RS:
                for feat in np.nonzero(ov)[0]:
                    cmap += [feat] * int(np.ceil(ov[feat] / RS))
            assert len(cmap) <= NF, (c, len(cmap))
            cmap += [0] * (NF - len(cmap))
            return np.array(cmap, np.int64)

        gP = cells_for(npos[c], RpI, RpS)
        gM = cells_for(nneg[c], RmI, RmS)

        # slot assignment: (round, partition) -> slot index
        s1 = np.zeros((NF, R), np.float32)    # -t (tame) / P (wild)
        s2 = np.zeros((NF, R), np.float32)    # +t (tame min, for ACT) / -Q
        mval = np.zeros((NF, R), np.float32)  # mask value (P^2 or 1)
        mjl = np.zeros((NF, R), np.int64)     # local j
        used = np.zeros((NF, R), bool)

        # round index lists per (tile, op)
        r_idx = {key: [q for q in range(R) if rounds[q] == key]
                 for key in set(rounds)}

        def place(slot_ids, id_rounds_, sp_rounds_, gmap):
            """slots of one sign (sorted by feature): fill identity rounds
            first (partition = feature), then spill cells."""
            order_f = np.argsort(f[slot_ids], kind="stable")
            sids = slot_ids[order_f]
            feats = f[sids]
            # identity placement
            taken = {}
            spill = []
            for sid, feat in zip(sids, feats):
                nid = taken.get(feat, 0)
                if nid < len(id_rounds_):
                    rr = id_rounds_[nid]
                    _set(rr, feat, sid)
                    taken[feat] = nid + 1
                else:
                    spill.append(sid)
            # spill placement
            if spill:
                cell_of = {}
                for p_, feat in enumerate(gmap):
                    cell_of.setdefault(feat, []).append(p_)
                fill = {}
                for sid in spill:
                    feat = f[sid]
                    cells = cell_of.get(feat)
                    assert cells, (c, feat)
                    n_ = fill.get(feat, 0)
                    ci, ri = n_ % len(cells), n_ // len(cells)
                    assert ri < len(sp_rounds_), (c, feat, n_)
                    _set(sp_rounds_[ri], cells[ci], sid)
                    fill[feat] = n_ + 1

        def _set(rr, p_, sid):
            assert not used[p_, rr], (c, rr, p_)
            used[p_, rr] = True
            if wild[sid]:
                s1[p_, rr] = P[sid]
                s2[p_, rr] = -Q[sid]
                mval[p_, rr] = 1.0
            else:
                s1[p_, rr] = -t[sid]
                s2[p_, rr] = t[sid]
                mval[p_, rr] = P[sid] * P[sid]
            mjl[p_, rr] = jl[sid]

        sel_p = np.nonzero(cs & tame & pos)[0]
        sel_m = np.nonzero(cs & tame & neg)[0]
        sel_w = np.nonzero(cs & wild)[0]
        place(sel_p, r_idx.get((0, OP_MAX), []), r_idx.get((1, OP_MAX), []), gP)
        place(sel_m, r_idx.get((0, OP_MIN), []), r_idx.get((2, OP_MIN), []), gM)
        # wild: identity tile, partition = feature, rounds (0, OP_FREE)
        wr = r_idx.get((0, OP_FREE), [])
        wtaken = {}
        for sid in sel_w:
            feat = f[sid]
            n_ = wtaken.get(feat, 0)
            assert n_ < len(wr), (c, feat)
            _set(wr[n_], feat, sid)
            wtaken[feat] = n_ + 1

        # resident tiles [128, 3*512] fp16: identity, spill+, spill-
        XT = np.ascontiguousarray(X.T)       # [128, 512]
        xt = np.concatenate([XT, XT[gP], XT[gM]], axis=1).astype(_NP16)

        # pq [128, 2R] fp32
        pq = np.concatenate([s1, s2], axis=1).astype(np.float32)

        # masks [128, R*64] fp16
        masks = np.zeros((NF, R, JC), np.float32)
        pp, rr_ = np.nonzero(used)
        masks[pp, rr_, mjl[pp, rr_]] = mval[pp, rr_]
        masks = np.ascontiguousarray(masks.reshape(NF, R * JC)).astype(_NP16)

        fv = np.zeros((JC, 1), np.float32)
        fv[:, 0] = Fvec[c * JC:(c + 1) * JC]
        in_maps.append({
            "xt": np.ascontiguousarray(xt),
            "pq": np.ascontiguousarray(pq),
            "masks": masks,
            "fvec": np.ascontiguousarray(fv.astype(_NP16)),
        })
    return schedule, in_maps


# ---------------------------------------------------------------- device IR
def _build_program(schedule, legalize=True):
    R = schedule["R"]
    rounds = schedule["rounds"]
    paths = schedule["paths"]

    nc = bass.Bass(enable_asserts=False)
    xt_d = nc.dram_tensor("xt", [NF, 3 * M], _DT16, kind="ExternalInput")
    pq_d = nc.dram_tensor("pq", [NF, 2 * R], _DT, kind="ExternalInput")
    mk_d = nc.dram_tensor("masks", [NF, R * JC], _DT16, kind="ExternalInput")
    fv_d = nc.dram_tensor("fvec", [JC, 1], _DT16, kind="ExternalInput")
    h_d = nc.dram_tensor("h_out", [1, M], _DT, kind="ExternalOutput")

    AF = mybir.ActivationFunctionType
    ALU = mybir.AluOpType
    MK_SPLIT = min(6, R)     # masks DMA chunk boundary (rounds)

    with tile.TileContext(nc) as tc:
        with (
            tc.tile_pool(name="consts", bufs=1) as consts,
            tc.tile_pool(name="up", bufs=4) as up,
            tc.tile_pool(name="r2p", bufs=6) as r2p,
            tc.tile_pool(name="outp", bufs=1) as outp,
            tc.tile_pool(name="psum", bufs=1, space="PSUM") as psum,
        ):
            pq_sb = consts.tile([NF, 2 * R], _DT)
            nc.scalar.dma_start(pq_sb[:], pq_d[:])
            xt_sb = consts.tile([NF, 3 * M], _DT16)
            nc.sync.dma_start(xt_sb[:, 0:M], xt_d[:, 0:M])
            mk_sb = consts.tile([NF, R * JC], _DT16)
            nc.scalar.dma_start(
                mk_sb[:, 0:MK_SPLIT * JC], mk_d[:, 0:MK_SPLIT * JC]
            )
            fv_sb = consts.tile([JC, 1], _DT16)
            nc.scalar.dma_start(fv_sb[:], fv_d[:])
            nc.sync.dma_start(xt_sb[:, M:3 * M], xt_d[:, M:3 * M])
            if MK_SPLIT < R:
                nc.scalar.dma_start(
                    mk_sb[:, MK_SPLIT * JC:], mk_d[:, MK_SPLIT * JC:]
                )

            e_ps = psum.tile([JC, M], _DT)
            for r in range(R):
                tl, op = rounds[r]
                relu_eng, sq_eng = paths[r]
                x_ap = xt_sb[:, tl * M:(tl + 1) * M]
                s1 = pq_sb[:, r:r + 1]
                s2 = pq_sb[:, R + r:R + r + 1]
                u = up.tile([NF, M], _DT16)
                if relu_eng == "dve":
                    if op == OP_MAX:
                        nc.vector.tensor_scalar(
                            u[:], x_ap, s1, 0.0, ALU.add, ALU.max)
                    elif op == OP_MIN:
                        nc.vector.tensor_scalar(
                            u[:], x_ap, s1, 0.0, ALU.add, ALU.min)
                    else:
                        nc.vector.tensor_scalar(
                            u[:], x_ap, s1, s2, ALU.mult, ALU.add)
                else:
                    if op == OP_MAX:
                        nc.scalar.activation(u[:], x_ap, AF.Relu, bias=s1)
                    elif op == OP_MIN:
                        nc.scalar.activation(
                            u[:], x_ap, AF.Relu, bias=s2, scale=-1.0)
                    else:
                        nc.scalar.activation(
                            u[:], x_ap, AF.Relu, bias=s2, scale=s1)
                r2 = r2p.tile([NF, M], _DT16)
                if sq_eng == "dve":
                    nc.vector.tensor_tensor(r2[:], u[:], u[:], ALU.mult)
                elif sq_eng == "pool":
                    nc.gpsimd.tensor_tensor(r2[:], u[:], u[:], ALU.mult)
                else:
                    nc.scalar.activation(r2[:], u[:], AF.Square)
                nc.tensor.matmul(
                    e_ps[:], mk_sb[:, r * JC:(r + 1) * JC], r2[:],
                    start=(r == 0), stop=(r == R - 1),
                )

            delta = outp.tile([JC, M], _DT16)
            nc.scalar.activation(delta[:], e_ps[:], AF.Exp, scale=-1.0)
            h_ps = psum.tile([1, M], _DT)
            nc.tensor.matmul(h_ps[:], fv_sb[:], delta[:], start=True, stop=True)
            h_sb = outp.tile([1, M], _DT)
            nc.vector.tensor_copy(h_sb[:], h_ps[:])
            nc.sync.dma_start(h_d[:], h_sb[:])
    if legalize:
        _legalize_waits(nc)
    return nc


# ---------------------------------------------------------------- profiling
def _install_ntff_shim():
    import sys
    import types

    if "antenv.axon_hooks" in sys.modules:
        return
    from trn_agent_boot.trn_boot import _ntff_profile_via_ctypes

    hook = _ntff_profile_via_ctypes("/opt/axon/libaxon_pjrt.so")
    mod = types.ModuleType("antenv.axon_hooks")
    mod.get_axon_ntff_profile_hook = lambda: hook
    mod.set_axon_ntff_profile_hook = lambda h: None
    sys.modules["antenv.axon_hooks"] = mod


# ---------------------------------------------------------------- entrypoint
def kernel(X, A_vals, V, W, Fvec, A_rows, A_cols, _want_trace=False):
    if _want_trace:
        _install_ntff_shim()
    schedule, in_maps = _prepare(X, A_vals, V, W, Fvec, A_rows, A_cols)
    nc = _build_program(schedule)
    res = run_bass_kernel_spmd(
        nc, in_maps, core_ids=list(range(NCORES)), trace=_want_trace
    )
    H = np.zeros(M, dtype=np.float32)
    for c in range(NCORES):
        H += res.results[c]["h_out"][0].astype(np.float32)
    kernel.last_result = res
    return H.astype(np.float32)
